# revision 6
# baseline (speedup 1.0000x reference)
"""log_matmul_exp(x, A) on 8 TRN2 NeuronCores via fp8 DoubleRow matmuls.

out[n, e] = logsumexp_d(x[n, d] + A[d, e]) = log(exp(x) @ exp(A))

Sharding: 4 shards of N x 2 shards of E. Per core M=1024, K=1024, N=2048.

Numerics (validated offline, rel err ~1.9e-3 vs 2e-2 budget):
- Host shifts x by (max(x)-5.3) and A by (max(A)-5.3) so exp() peaks at
  e^5.3=200 < 240 (TRN e4m3 max normal); ships bf16.
- ACT computes exp with fp8e4 output (quant noise ~3.6% RMS per operand,
  averaged down by the 1024-deep sum to ~5e-4 in the log output).
- PE runs fp8 DoubleRow matmuls: operands [128, 2, F] contract 256/instr.
- DVE computes ln via the Mitchell bit trick: ln(s) ~ int_bits(s) *
  (ln2/2^23) + (mu - 127)*ln2, one tensor_scalar (mult, add) per PSUM
  group, fused with the shift-restore constant C and fp16 output cast.
  This keeps ln OFF the ACT engine, whose exp work is already ~23us.

Engine budget per core: PE 27.3us (157 TF/s fp8), ACT ~23us (exp 3M elem),
DVE ~18us (Mitchell on 2M), DMA 10MB ~28us. Everything overlaps.
"""

import os
import sys

import numpy as np

for _p in ("/opt/trn_rl_repo", "/root/.axon_site/_ro/trn_rl_repo"):
    if os.path.isdir(_p) and _p not in sys.path:
        sys.path.insert(0, _p)

P = 128
D = 1024
N_FULL = 4096
E_FULL = 4096
GRID_N = 4
GRID_E = 2
N_CORES = GRID_N * GRID_E
ML = N_FULL // GRID_N  # 1024 local output rows
EL = E_FULL // GRID_E  # 2048 local output cols
KQ = D // (2 * P)  # 4 double-row contraction chunks of 256
MT = ML // P  # 8 row tiles
NT = 512  # matmul moving free dim (one PSUM bank of fp32)
ET = EL // NT  # 4 col tiles

SHIFT_HEADROOM = 5.3  # exp(max - shift) = e^5.3 = 200 < 240 (TRN e4m3 max)
MITCHELL_MU = 0.043  # mantissa-correction bias for the bit-trick log
LN2 = 0.6931471805599453
MITCHELL_MUL = LN2 / (1 << 23)

# False -> exact Ln on the ACT engine (fallback; ACT becomes bottleneck)
MITCHELL_ON_DVE = True

_cache: dict = {}


def _patch_ldw_opt():
    """Enable walrus's LDWEIGHTS optimization (dedups/hides redundant weight
    loads). concourse hardcodes --enable-ldw-opt=false; our inner loops reuse
    each stationary tile across 4 matmuls, so the reload elision matters."""
    if _cache.get("ldw_patched"):
        return
    from concourse import bass_utils

    orig = bass_utils.run_command

    def patched(argv, **kwargs):
        argv = [
            a.replace("--enable-ldw-opt=false", "--enable-ldw-opt=true")
            if isinstance(a, str)
            else a
            for a in argv
        ]
        return orig(argv, **kwargs)

    bass_utils.run_command = patched
    _cache["ldw_patched"] = True


def _build():
    import concourse.tile as tile
    from concourse import bacc, mybir

    AF = mybir.ActivationFunctionType
    ALU = mybir.AluOpType
    DR = mybir.MatmulPerfMode.DoubleRow
    f32 = mybir.dt.float32
    f16 = mybir.dt.float16
    i32 = mybir.dt.int32
    bf16 = mybir.dt.bfloat16
    fp8 = mybir.dt.float8e4

    # Bacc (not raw Bass): its compile() runs generate_event_semaphores,
    # which splits multi-wait instructions to satisfy the 1-wait-per-
    # instruction hardware constraint that walrus codegen enforces.
    nc = bacc.Bacc(
        "TRN2",
        target_bir_lowering=False,
        debug=False,
        num_devices=N_CORES,
        num_swdge_queues=4,
        dynamic_dma_scratch_size=256,
    )
    xt = nc.dram_tensor("xt", [D, ML], bf16, kind="ExternalInput")
    a = nc.dram_tensor("a", [D, EL], bf16, kind="ExternalInput")
    cvec = nc.dram_tensor("cvec", [P, 1], f32, kind="ExternalInput")
    out = nc.dram_tensor("out", [ML, EL], f16, kind="ExternalOutput")

    # DoubleRow pair layout: SBUF tile [p, i, f] = src[kq*256 + i*128 + p, f]
    xt4 = xt[:].rearrange("(kq i p) m -> p kq i m", p=P, i=2)  # [P, KQ, 2, ML]
    a4 = a[:].rearrange("(kq i p) e -> p kq i e", p=P, i=2)  # [P, KQ, 2, EL]

    with tile.TileContext(nc) as tc:
        with (
            tc.tile_pool(name="persist", bufs=1) as persist,
            tc.tile_pool(name="outp", bufs=4) as outp,
            tc.tile_pool(name="psum", bufs=2, space="PSUM") as psum_pool,
            tc.tile_pool(name="stage", bufs=8) as stage,
        ):
            # PE warm-up: dummy fp8 DoubleRow matmuls run while the first
            # inputs stream in, so the HAM clock gate reaches 8/8 (2.4 GHz)
            # before the real matmuls start (cold is 2x slower).
            wm = persist.tile([P, 2 * NT], fp8, tag="warm")
            nc.vector.memset(wm[:], 1.0)
            wm3 = wm[:].rearrange("p (i f) -> p i f", i=2)
            wps = psum_pool.tile([P, ET * NT], f32, tag="ps", name="warm_ps")
            for _ in range(20):
                nc.tensor.matmul(
                    wps[:, :NT],
                    lhsT=wm3[:, :, :P],
                    rhs=wm3,
                    start=True,
                    stop=True,
                    perf_mode=DR,
                )

            cv = persist.tile([P, 1], f32, tag="cv")
            nc.sync.dma_start(cv[:], cvec[:])

            # Whole-chunk loads (DMA issue costs ~0.6us per instruction, so
            # fewer/bigger transfers win); piecewise exp on the first chunk
            # only, so the first matmul starts early.
            ex = []
            ea = []
            for kq in range(KQ):
                st = stage.tile([P, 2 * ML], bf16, tag="stx")
                nc.sync.dma_start(
                    st[:].rearrange("p (i m) -> p i m", i=2), xt4[:, kq]
                )
                t = persist.tile([P, 2 * ML], fp8, tag=f"ex{kq}")
                if kq == 0:
                    for q in range(0, 2 * ML, NT):
                        nc.scalar.activation(
                            t[:, q : q + NT], st[:, q : q + NT], AF.Exp
                        )
                else:
                    nc.scalar.activation(t[:], st[:], AF.Exp)
                ex.append(t[:].rearrange("p (i m) -> p i m", i=2))
                su = stage.tile([P, 2 * EL], bf16, tag="sta")
                nc.sync.dma_start(
                    su[:].rearrange("p (i e) -> p i e", i=2), a4[:, kq]
                )
                u = persist.tile([P, 2 * EL], fp8, tag=f"ea{kq}")
                if kq == 0:
                    for q in range(0, 2 * EL, 2 * NT):
                        nc.scalar.activation(
                            u[:, q : q + 2 * NT], su[:, q : q + 2 * NT], AF.Exp
                        )
                else:
                    nc.scalar.activation(u[:], su[:], AF.Exp)
                ea.append(u[:].rearrange("p (i e) -> p i e", i=2))

            # Per row tile: 4 PSUM banks accumulate the 4 col tiles across
            # the 4 contraction chunks; kq OUTER / nt INNER so 4 consecutive
            # matmuls share the same stationary weight tile. The epilogue
            # reads the whole 4-bank group as one [P, 2048] AP.
            for mt in range(MT):
                ps = psum_pool.tile([P, ET * NT], f32, tag="ps", name=f"ps_{mt}")
                for kq in range(KQ):
                    lhs = ex[kq][:, :, mt * P : (mt + 1) * P]
                    for nt in range(ET):
                        nc.tensor.matmul(
                            ps[:, nt * NT : (nt + 1) * NT],
                            lhsT=lhs,
                            rhs=ea[kq][:, :, nt * NT : (nt + 1) * NT],
                            start=(kq == 0),
                            stop=(kq == KQ - 1),
                            perf_mode=DR,
                        )
                ob = outp.tile([P, ET * NT], f16, tag="ob", name=f"ob_{mt}")
                if MITCHELL_ON_DVE:
                    nc.vector.tensor_scalar(
                        ob[:],
                        ps[:].bitcast(i32),
                        MITCHELL_MUL,
                        cv[:],
                        ALU.mult,
                        ALU.add,
                    )
                else:
                    nc.scalar.activation(ob[:], ps[:], AF.Ln, scale=cv[:])
                nc.sync.dma_start(out[mt * P : (mt + 1) * P, :], ob[:])
    nc.compile()
    return nc


def _shard_inputs(x: np.ndarray, A: np.ndarray) -> list[dict]:
    import ml_dtypes

    bf = ml_dtypes.bfloat16
    x = np.asarray(x, dtype=np.float32)
    A = np.asarray(A, dtype=np.float32)
    sx = float(x.max()) - SHIFT_HEADROOM
    sa = float(A.max()) - SHIFT_HEADROOM
    C = sx + sa
    if MITCHELL_ON_DVE:
        cval = (MITCHELL_MU - 127.0) * LN2 + C
    else:
        cval = float(np.exp(C))  # Ln(s * e^C) = ln(s) + C via the scale port
    cvec = np.full((P, 1), cval, dtype=np.float32)
    xT = np.ascontiguousarray((x - sx).T.astype(bf))  # (D, N)
    Ab = (A - sa).astype(bf)  # (D, E)
    in_maps = []
    for c in range(N_CORES):
        i, j = divmod(c, GRID_E)
        in_maps.append(
            {
                "xt": np.ascontiguousarray(xT[:, i * ML : (i + 1) * ML]),
                "a": np.ascontiguousarray(Ab[:, j * EL : (j + 1) * EL]),
                "cvec": cvec,
            }
        )
    return in_maps


def _run(x: np.ndarray, A: np.ndarray, trace: bool = False):
    from concourse import bass_utils

    # NOTE: _patch_ldw_opt (walrus --enable-ldw-opt=true) is NOT applied:
    # walrus rejects DoubleRow InstLdweights under the LDW optimization.
    nc = _cache.get("nc")
    if nc is None:
        nc = _build()
        _cache["nc"] = nc

    in_maps = _shard_inputs(np.asarray(x), np.asarray(A))
    res = bass_utils.run_bass_kernel_spmd(
        nc, in_maps, list(range(N_CORES)), trace=trace
    )
    out = np.empty((N_FULL, E_FULL), dtype=np.float32)
    for c in range(N_CORES):
        i, j = divmod(c, GRID_E)
        out[i * ML : (i + 1) * ML, j * EL : (j + 1) * EL] = res.results[c][
            "out"
        ].astype(np.float32)
    return out, res


def kernel(x: np.ndarray, A: np.ndarray) -> np.ndarray:
    out, _ = _run(x, A, trace=False)
    return out


# revision 19
# speedup vs baseline: 1.2394x; 1.2394x over previous
"""log_matmul_exp(x, A) on 8 TRN2 NeuronCores via fp8 DoubleRow matmuls.

out[n, e] = logsumexp_d(x[n, d] + A[d, e]) = log(exp(x) @ exp(A))

Sharding: 4 shards of N x 2 shards of E. Per core M=1024, K=1024, N=2048.

Numerics (validated offline + on HW, rel err ~3e-3 vs 2e-2 budget):
- Host shifts x by (max(x)-5.3) and A by (max(A)-5.3) so exp() peaks at
  e^5.3=200 < 240 (TRN e4m3 max normal), then quantizes to int8 with step
  5.32/127 (~1.2% RMS exp noise, under fp8's 3.6%). Halves input DMA vs
  bf16 and the ACT engine dequantizes for free via the activation scale
  port (verified bit-exact on HW).
- ACT computes exp(q * QS) with fp8e4 output.
- PE runs fp8 DoubleRow matmuls: operands [128, 2, F] contract 256/instr
  at 216 ns per [128x512] tile (157 TF/s, measured).
- DVE computes ln via the Mitchell bit trick: ln(s) ~ int_bits(s) *
  (ln2/2^23) + (mu - 127)*ln2 + C, one tensor_scalar (mult, add) per
  PSUM bank, writing fp16. Keeps ln OFF the ACT engine whose exp chain
  is the ramp-limiting resource.

Structure: E-striped. x (1MB int8) loads as ONE image DMA; A arrives as 4
column-stripe images (0.5MB each). Each A stripe enables a full
[1024 x 512] output stripe (8 mt x 4 kq matmuls, kq-outer so per-kq exp
pieces feed the PE just-in-time). 8 PSUM banks = one stripe of row tiles.
Host pre-swizzles both inputs into exact SBUF images so every DMA line is
partition-contiguous (2-8KB runs).

Engine budget per core: PE 27.6us, ACT ~22us exp chain, DVE ~23us
Mitchell, DMA 3MB in + 4MB out ~20us. Ramp ~14us (preamble + first data
+ first exp pieces), then PE-bound.
"""

import os
import sys

import numpy as np

for _p in ("/opt/trn_rl_repo", "/root/.axon_site/_ro/trn_rl_repo"):
    if os.path.isdir(_p) and _p not in sys.path:
        sys.path.insert(0, _p)

P = 128
D = 1024
N_FULL = 4096
E_FULL = 4096
GRID_N = 4
GRID_E = 2
N_CORES = GRID_N * GRID_E
ML = N_FULL // GRID_N  # 1024 local output rows
EL = E_FULL // GRID_E  # 2048 local output cols
KQ = D // (2 * P)  # 4 double-row contraction chunks of 256
MT = ML // P  # 8 row tiles
NT = 512  # matmul moving free dim (one PSUM bank of fp32)
NS = EL // NT  # 4 output col stripes
N_WARM = 12

SHIFT_HEADROOM = 5.3  # exp(max - shift) = e^5.3 = 200 < 240 (TRN e4m3 max)
QS = 5.32 / 127.0  # int8 quant step (shared by x and A; compile-time const)
MITCHELL_MU = 0.043  # mantissa-correction bias for the bit-trick log
LN2 = 0.6931471805599453
MITCHELL_MUL = LN2 / (1 << 23)

_cache: dict = {}


def _build():
    import concourse.tile as tile
    from concourse import bacc, mybir

    AF = mybir.ActivationFunctionType
    ALU = mybir.AluOpType
    DR = mybir.MatmulPerfMode.DoubleRow
    f32 = mybir.dt.float32
    f16 = mybir.dt.float16
    i32 = mybir.dt.int32
    i8 = mybir.dt.int8
    fp8 = mybir.dt.float8e4

    nc = bacc.Bacc(
        "TRN2",
        target_bir_lowering=False,
        debug=False,
        num_devices=N_CORES,
        num_swdge_queues=4,
        dynamic_dma_scratch_size=256,
    )
    # Host-pre-swizzled SBUF images (see _shard_inputs):
    #   xq[p, kq*2048 + i*1024 + m] = x_int8[d = kq*256 + i*128 + p, m]
    #   aq[p, s*4096 + kq*1024 + i*512 + e] = A_int8[kq*256+i*128+p, s*512+e]
    xq = nc.dram_tensor("xq", [P, KQ * 2 * ML], i8, kind="ExternalInput")
    aq = nc.dram_tensor("aq", [P, NS * KQ * 2 * NT], i8, kind="ExternalInput")
    cvec = nc.dram_tensor("cvec", [P, 1], f32, kind="ExternalInput")
    cvec2 = nc.dram_tensor("cvec2", [P, 1], f32, kind="ExternalInput")
    out = nc.dram_tensor("out", [ML, EL], f16, kind="ExternalOutput")

    with tile.TileContext(nc) as tc:
        with (
            tc.tile_pool(name="persist", bufs=1) as persist,
            tc.tile_pool(name="eap", bufs=3) as eap,
            tc.tile_pool(name="ost", bufs=2) as ost,
            tc.tile_pool(name="psum", bufs=8, space="PSUM") as psum_pool,
            tc.tile_pool(name="stage", bufs=3) as stage,
        ):
            # PE warm-up: dummy fp8 DoubleRow matmuls bridge the input-load
            # window so the HAM clock gate reaches 8/8 (2.4 GHz) before the
            # real matmuls start (cold is 2x slower).
            wm = persist.tile([P, 2 * NT], fp8, tag="warm")
            nc.vector.memset(wm[:], 1.0)
            wm3 = wm[:].rearrange("p (i f) -> p i f", i=2)
            wps = psum_pool.tile([P, NT], f32, tag="ps", name="warm_ps")
            for _ in range(N_WARM):
                nc.tensor.matmul(
                    wps[:],
                    lhsT=wm3[:, :, :P],
                    rhs=wm3,
                    start=True,
                    stop=True,
                    perf_mode=DR,
                )

            cv = persist.tile([P, 1], f32, tag="cv")
            nc.sync.dma_start(cv[:], cvec[:])
            cv2 = persist.tile([P, 1], f32, tag="cv2")
            nc.sync.dma_start(cv2[:], cvec2[:])

            # Input DMAs. Outstanding transfers share HW queue bandwidth
            # round-robin, so small early pieces finish fast while later
            # issues (0.64us apart on the Sync queue) trickle in behind:
            # a0/a1 and x arrive as per-kq pieces ordered by exp-chain
            # deadline; a2/a3 are whole stripes needed much later.
            asb = []
            for s in range(NS):
                t = stage.tile([P, KQ * 2 * NT], i8, tag="sta", name=f"as{s}")
                asb.append(t)
            AW = 2 * NT  # A kq-piece width
            XW = 2 * ML  # x kq-piece width
            for kq in range(KQ):
                nc.sync.dma_start(
                    asb[0][:, kq * AW : (kq + 1) * AW],
                    aq[:, kq * AW : (kq + 1) * AW],
                )
            xs = persist.tile([P, KQ * XW], i8, tag="xs")
            for kq in range(KQ):
                nc.sync.dma_start(
                    xs[:, kq * XW : (kq + 1) * XW],
                    xq[:, kq * XW : (kq + 1) * XW],
                )
            for kq in range(KQ):
                nc.sync.dma_start(
                    asb[1][:, kq * AW : (kq + 1) * AW],
                    aq[:, KQ * AW + kq * AW : KQ * AW + (kq + 1) * AW],
                )
            for s in range(2, NS):
                nc.sync.dma_start(
                    asb[s][:], aq[:, s * KQ * AW : (s + 1) * KQ * AW]
                )

            # exp chain on ACT (the ramp-limiting resource). ea0/ea1 in
            # per-kq pieces so stripe-0/1 matmuls start as pieces land;
            # x pieces between them; ea2/ea3 in halves (less overhead).
            XP = 2 * ML  # 2048 cols per x kq-piece
            AP_ = 2 * NT  # 1024 cols per A kq-piece
            exf = persist.tile([P, KQ * XP], fp8, tag="exf")
            eat = []
            for s in range(NS):
                t = eap.tile([P, KQ * AP_], fp8, tag="ea", name=f"ea{s}")
                eat.append(t)

            def exp_a(s, pieces):
                w = KQ * AP_ // pieces
                for q in range(pieces):
                    nc.scalar.activation(
                        eat[s][:, q * w : (q + 1) * w],
                        asb[s][:, q * w : (q + 1) * w],
                        AF.Exp,
                        scale=QS,
                    )

            exp_a(0, KQ)
            for kq in range(KQ):
                nc.scalar.activation(
                    exf[:, kq * XP : (kq + 1) * XP],
                    xs[:, kq * XP : (kq + 1) * XP],
                    AF.Exp,
                    scale=QS,
                )
            exp_a(1, KQ)
            exp_a(2, 1)
            exp_a(3, 1)

            ex3 = exf[:].rearrange("p (kq i m) -> p kq i m", kq=KQ, i=2)

            # Stripes: kq-outer / mt-inner; 8 PSUM banks hold one stripe's
            # row tiles. Per-bank epilogues keep the bank-recycle chain
            # fine-grained so the next stripe's matmuls never wait long.
            # The tail epilogues (late banks of the last two stripes) run
            # exact Ln on ACT — idle after its exp chain — in parallel with
            # DVE's Mitchell, shrinking the critical tail.
            for s in range(NS):
                ea3 = eat[s][:].rearrange("p (kq i e) -> p kq i e", kq=KQ, i=2)
                pss = [
                    psum_pool.tile([P, NT], f32, tag="ps", name=f"ps_{s}_{mt}")
                    for mt in range(MT)
                ]
                for kq in range(KQ):
                    rhs = ea3[:, kq]
                    for mt in range(MT):
                        nc.tensor.matmul(
                            pss[mt][:],
                            lhsT=ex3[:, kq, :, mt * P : (mt + 1) * P],
                            rhs=rhs,
                            start=(kq == 0),
                            stop=(kq == KQ - 1),
                            perf_mode=DR,
                        )
                ob = ost.tile([P, MT * NT], f16, tag="ob", name=f"ob_{s}")
                ov = out[:, s * NT : (s + 1) * NT].rearrange(
                    "(mt p) e -> p mt e", p=P
                )
                ob3 = ob[:].rearrange("p (mt e) -> p mt e", mt=MT)
                on_act = MT - 2 if s == NS - 2 else (MT // 2 if s == NS - 1 else MT)
                for mt in range(MT):
                    obm = ob[:, mt * NT : (mt + 1) * NT]
                    if mt >= on_act:
                        nc.scalar.activation(
                            obm, pss[mt][:], AF.Ln, scale=cv2[:]
                        )
                    else:
                        nc.vector.tensor_scalar(
                            obm,
                            pss[mt][:].bitcast(i32),
                            MITCHELL_MUL,
                            cv[:],
                            ALU.mult,
                            ALU.add,
                        )
                    if mt == MT // 2 - 1:
                        nc.sync.dma_start(ov[:, : MT // 2], ob3[:, : MT // 2])
                nc.sync.dma_start(ov[:, MT // 2 :], ob3[:, MT // 2 :])
    nc.compile()
    return nc


def _quant_int8(v: np.ndarray, shift: float) -> np.ndarray:
    q = np.rint((v - shift) * (1.0 / QS))
    return np.clip(q, -128, 127).astype(np.int8)


def _shard_inputs(x: np.ndarray, A: np.ndarray) -> list[dict]:
    x = np.asarray(x, dtype=np.float32)
    A = np.asarray(A, dtype=np.float32)
    sx = float(x.max()) - SHIFT_HEADROOM
    sa = float(A.max()) - SHIFT_HEADROOM
    C = sx + sa
    cvec = np.full(
        (P, 1), (MITCHELL_MU - 127.0) * LN2 + C, dtype=np.float32
    )
    cvec2 = np.full((P, 1), np.exp(C), dtype=np.float32)
    xi = _quant_int8(x, sx)  # (N, D)
    ai = _quant_int8(A, sa)  # (D, E)
    in_maps = []
    for c in range(N_CORES):
        i, j = divmod(c, GRID_E)
        # x image: [D, ML] -> [kq, i2, p, m] -> [p, kq*i2*m]
        xs = np.ascontiguousarray(xi[i * ML : (i + 1) * ML, :].T)
        xim = (
            xs.reshape(KQ, 2, P, ML)
            .transpose(2, 0, 1, 3)
            .reshape(P, KQ * 2 * ML)
        )
        # A image: [D, EL] -> [kq, i2, p, s, e] -> [p, s*kq*i2*e]
        asd = ai[:, j * EL : (j + 1) * EL]
        aim = (
            asd.reshape(KQ, 2, P, NS, NT)
            .transpose(2, 3, 0, 1, 4)
            .reshape(P, NS * KQ * 2 * NT)
        )
        in_maps.append(
            {
                "xq": np.ascontiguousarray(xim),
                "aq": np.ascontiguousarray(aim),
                "cvec": cvec,
                "cvec2": cvec2,
            }
        )
    return in_maps


def _run(x: np.ndarray, A: np.ndarray, trace: bool = False):
    from concourse import bass_utils

    nc = _cache.get("nc")
    if nc is None:
        nc = _build()
        _cache["nc"] = nc

    in_maps = _shard_inputs(np.asarray(x), np.asarray(A))
    res = bass_utils.run_bass_kernel_spmd(
        nc, in_maps, list(range(N_CORES)), trace=trace
    )
    out = np.empty((N_FULL, E_FULL), dtype=np.float32)
    for c in range(N_CORES):
        i, j = divmod(c, GRID_E)
        out[i * ML : (i + 1) * ML, j * EL : (j + 1) * EL] = res.results[c][
            "out"
        ].astype(np.float32)
    return out, res


def kernel(x: np.ndarray, A: np.ndarray) -> np.ndarray:
    out, _ = _run(x, A, trace=False)
    return out


# revision 22
# speedup vs baseline: 1.3363x; 1.0781x over previous
"""log_matmul_exp(x, A) on 8 TRN2 NeuronCores via fp8 DoubleRow matmuls.

out[n, e] = logsumexp_d(x[n, d] + A[d, e]) = log(exp(x) @ exp(A))

Sharding: 4 shards of N x 2 shards of E. Per core M=1024, K=1024, N=2048.

Numerics (validated offline + on HW, rel err ~3e-3 vs 2e-2 budget):
- Host shifts x by (max(x)-5.3) and A by (max(A)-5.3) so exp() peaks at
  e^5.3=200 < 240 (TRN e4m3 max normal), then quantizes to int8 with step
  5.32/127 (~1.2% RMS exp noise, under fp8's 3.6%). Halves input DMA vs
  bf16 and the ACT engine dequantizes for free via the activation scale
  port (verified bit-exact on HW).
- ACT computes exp(q * QS) with fp8e4 output.
- PE runs fp8 DoubleRow matmuls: operands [128, 2, F] contract 256/instr
  at 216 ns per [128x512] tile (157 TF/s, measured).
- DVE computes ln via the Mitchell bit trick: ln(s) ~ int_bits(s) *
  (ln2/2^23) + (mu - 127)*ln2 + C, one tensor_scalar (mult, add) per
  PSUM bank, writing fp16. Keeps ln OFF the ACT engine whose exp chain
  is the ramp-limiting resource.

Structure: E-striped. x (1MB int8) loads as ONE image DMA; A arrives as 4
column-stripe images (0.5MB each). Each A stripe enables a full
[1024 x 512] output stripe (8 mt x 4 kq matmuls, kq-outer so per-kq exp
pieces feed the PE just-in-time). 8 PSUM banks = one stripe of row tiles.
Host pre-swizzles both inputs into exact SBUF images so every DMA line is
partition-contiguous (2-8KB runs).

Engine budget per core: PE 27.6us, ACT ~22us exp chain, DVE ~23us
Mitchell, DMA 3MB in + 4MB out ~20us. Ramp ~14us (preamble + first data
+ first exp pieces), then PE-bound.
"""

import os
import sys

import numpy as np

for _p in ("/opt/trn_rl_repo", "/root/.axon_site/_ro/trn_rl_repo"):
    if os.path.isdir(_p) and _p not in sys.path:
        sys.path.insert(0, _p)

P = 128
D = 1024
N_FULL = 4096
E_FULL = 4096
GRID_N = 4
GRID_E = 2
N_CORES = GRID_N * GRID_E
ML = N_FULL // GRID_N  # 1024 local output rows
EL = E_FULL // GRID_E  # 2048 local output cols
KQ = D // (2 * P)  # 4 double-row contraction chunks of 256
MT = ML // P  # 8 row tiles
NT = 512  # matmul moving free dim (one PSUM bank of fp32)
NS = EL // NT  # 4 output col stripes
N_WARM = 20

SHIFT_HEADROOM = 5.3  # exp(max - shift) = e^5.3 = 200 < 240 (TRN e4m3 max)
QS = 5.32 / 127.0  # int8 quant step (shared by x and A; compile-time const)
MITCHELL_MU = 0.043  # mantissa-correction bias for the bit-trick log
LN2 = 0.6931471805599453
MITCHELL_MUL = LN2 / (1 << 23)

_cache: dict = {}


def _build():
    import concourse.tile as tile
    from concourse import bacc, mybir

    AF = mybir.ActivationFunctionType
    ALU = mybir.AluOpType
    DR = mybir.MatmulPerfMode.DoubleRow
    f32 = mybir.dt.float32
    f16 = mybir.dt.float16
    i32 = mybir.dt.int32
    i8 = mybir.dt.int8
    fp8 = mybir.dt.float8e4

    nc = bacc.Bacc(
        "TRN2",
        target_bir_lowering=False,
        debug=False,
        num_devices=N_CORES,
        num_swdge_queues=4,
        dynamic_dma_scratch_size=256,
    )
    # Host-pre-swizzled SBUF images (see _shard_inputs):
    #   xq[p, kq*2048 + i*1024 + m] = x_int8[d = kq*256 + i*128 + p, m]
    #   aq[p, s*4096 + kq*1024 + i*512 + e] = A_int8[kq*256+i*128+p, s*512+e]
    xq = nc.dram_tensor("xq", [P, KQ * 2 * ML], i8, kind="ExternalInput")
    aq = nc.dram_tensor("aq", [P, NS * KQ * 2 * NT], i8, kind="ExternalInput")
    cvec = nc.dram_tensor("cvec", [P, 1], f32, kind="ExternalInput")
    cvec2 = nc.dram_tensor("cvec2", [P, 1], f32, kind="ExternalInput")
    out = nc.dram_tensor("out", [ML, EL], f16, kind="ExternalOutput")

    with tile.TileContext(nc) as tc:
        with (
            tc.tile_pool(name="persist", bufs=1) as persist,
            tc.tile_pool(name="eap", bufs=3) as eap,
            tc.tile_pool(name="ost", bufs=2) as ost,
            tc.tile_pool(name="psum", bufs=8, space="PSUM") as psum_pool,
            tc.tile_pool(name="stage", bufs=3) as stage,
        ):
            # PE warm-up: dummy fp8 DoubleRow matmuls bridge the input-load
            # window so the HAM clock gate reaches 8/8 (2.4 GHz) before the
            # real matmuls start (cold is 2x slower).
            wm = persist.tile([P, 2 * NT], fp8, tag="warm")
            nc.vector.memset(wm[:], 1.0)
            wm3 = wm[:].rearrange("p (i f) -> p i f", i=2)
            wps = psum_pool.tile([P, NT], f32, tag="ps", name="warm_ps")
            for _ in range(N_WARM):
                nc.tensor.matmul(
                    wps[:],
                    lhsT=wm3[:, :, :P],
                    rhs=wm3,
                    start=True,
                    stop=True,
                    perf_mode=DR,
                )

            # Input DMAs. Outstanding transfers share HW queue bandwidth
            # round-robin, so small early pieces finish fast while later
            # issues (0.64us apart on the Sync queue) trickle in behind:
            # a0 and x arrive as per-kq pieces ordered by exp-chain
            # deadline; a1-a3 are whole stripes needed much later. The tiny
            # cvec loads are issued AFTER the critical inputs (they're only
            # needed by the first epilogue at ~24us).
            asb = []
            for s in range(NS):
                t = stage.tile([P, KQ * 2 * NT], i8, tag="sta", name=f"as{s}")
                asb.append(t)
            AW = 2 * NT  # A kq-piece width
            XW = 2 * ML  # x kq-piece width
            for kq in range(KQ):
                nc.sync.dma_start(
                    asb[0][:, kq * AW : (kq + 1) * AW],
                    aq[:, kq * AW : (kq + 1) * AW],
                )
            xs = persist.tile([P, KQ * XW], i8, tag="xs")
            for kq in range(KQ):
                nc.sync.dma_start(
                    xs[:, kq * XW : (kq + 1) * XW],
                    xq[:, kq * XW : (kq + 1) * XW],
                )
            cv = persist.tile([P, 1], f32, tag="cv")
            nc.sync.dma_start(cv[:], cvec[:])
            cv2 = persist.tile([P, 1], f32, tag="cv2")
            nc.sync.dma_start(cv2[:], cvec2[:])
            for s in range(1, NS):
                nc.sync.dma_start(
                    asb[s][:], aq[:, s * KQ * AW : (s + 1) * KQ * AW]
                )

            # exp chain on ACT (the ramp-limiting resource). ea0/ea1 in
            # per-kq pieces so stripe-0/1 matmuls start as pieces land;
            # x pieces between them; ea2/ea3 in halves (less overhead).
            XP = 2 * ML  # 2048 cols per x kq-piece
            AP_ = 2 * NT  # 1024 cols per A kq-piece
            exf = persist.tile([P, KQ * XP], fp8, tag="exf")
            eat = []
            for s in range(NS):
                t = eap.tile([P, KQ * AP_], fp8, tag="ea", name=f"ea{s}")
                eat.append(t)

            def exp_a(s, pieces):
                w = KQ * AP_ // pieces
                for q in range(pieces):
                    nc.scalar.activation(
                        eat[s][:, q * w : (q + 1) * w],
                        asb[s][:, q * w : (q + 1) * w],
                        AF.Exp,
                        scale=QS,
                    )

            exp_a(0, KQ)
            for kq in range(KQ):
                nc.scalar.activation(
                    exf[:, kq * XP : (kq + 1) * XP],
                    xs[:, kq * XP : (kq + 1) * XP],
                    AF.Exp,
                    scale=QS,
                )
            exp_a(1, KQ)
            exp_a(2, 1)
            exp_a(3, 1)

            ex3 = exf[:].rearrange("p (kq i m) -> p kq i m", kq=KQ, i=2)

            # Stripes: kq-outer / mt-inner; 8 PSUM banks hold one stripe's
            # row tiles. Per-bank epilogues keep the bank-recycle chain
            # fine-grained so the next stripe's matmuls never wait long.
            # The tail epilogues (late banks of the last two stripes) run
            # exact Ln on ACT — idle after its exp chain — in parallel with
            # DVE's Mitchell, shrinking the critical tail.
            for s in range(NS):
                ea3 = eat[s][:].rearrange("p (kq i e) -> p kq i e", kq=KQ, i=2)
                pss = [
                    psum_pool.tile([P, NT], f32, tag="ps", name=f"ps_{s}_{mt}")
                    for mt in range(MT)
                ]
                for kq in range(KQ):
                    rhs = ea3[:, kq]
                    for mt in range(MT):
                        nc.tensor.matmul(
                            pss[mt][:],
                            lhsT=ex3[:, kq, :, mt * P : (mt + 1) * P],
                            rhs=rhs,
                            start=(kq == 0),
                            stop=(kq == KQ - 1),
                            perf_mode=DR,
                        )
                ob = ost.tile([P, MT * NT], f16, tag="ob", name=f"ob_{s}")
                ov = out[:, s * NT : (s + 1) * NT].rearrange(
                    "(mt p) e -> p mt e", p=P
                )
                ob3 = ob[:].rearrange("p (mt e) -> p mt e", mt=MT)
                on_act = MT - 2 if s == NS - 2 else (MT // 2 if s == NS - 1 else MT)
                for mt in range(MT):
                    obm = ob[:, mt * NT : (mt + 1) * NT]
                    if mt >= on_act:
                        nc.scalar.activation(
                            obm, pss[mt][:], AF.Ln, scale=cv2[:]
                        )
                    else:
                        nc.vector.tensor_scalar(
                            obm,
                            pss[mt][:].bitcast(i32),
                            MITCHELL_MUL,
                            cv[:],
                            ALU.mult,
                            ALU.add,
                        )
                    # Last stripe: quarter DMAs issued per mt-pair so the
                    # final transfer (the tail) is only 256KB.
                    if s == NS - 1:
                        if mt % 2 == 1:
                            nc.sync.dma_start(
                                ov[:, mt - 1 : mt + 1], ob3[:, mt - 1 : mt + 1]
                            )
                    elif mt == MT // 2 - 1:
                        nc.sync.dma_start(ov[:, : MT // 2], ob3[:, : MT // 2])
                if s != NS - 1:
                    nc.sync.dma_start(ov[:, MT // 2 :], ob3[:, MT // 2 :])
    nc.compile()
    return nc


def _quant_int8(v: np.ndarray, shift: float) -> np.ndarray:
    q = np.rint((v - shift) * (1.0 / QS))
    return np.clip(q, -128, 127).astype(np.int8)


def _shard_inputs(x: np.ndarray, A: np.ndarray) -> list[dict]:
    x = np.asarray(x, dtype=np.float32)
    A = np.asarray(A, dtype=np.float32)
    sx = float(x.max()) - SHIFT_HEADROOM
    sa = float(A.max()) - SHIFT_HEADROOM
    C = sx + sa
    cvec = np.full(
        (P, 1), (MITCHELL_MU - 127.0) * LN2 + C, dtype=np.float32
    )
    cvec2 = np.full((P, 1), np.exp(C), dtype=np.float32)
    xi = _quant_int8(x, sx)  # (N, D)
    ai = _quant_int8(A, sa)  # (D, E)
    in_maps = []
    for c in range(N_CORES):
        i, j = divmod(c, GRID_E)
        # x image: [D, ML] -> [kq, i2, p, m] -> [p, kq*i2*m]
        xs = np.ascontiguousarray(xi[i * ML : (i + 1) * ML, :].T)
        xim = (
            xs.reshape(KQ, 2, P, ML)
            .transpose(2, 0, 1, 3)
            .reshape(P, KQ * 2 * ML)
        )
        # A image: [D, EL] -> [kq, i2, p, s, e] -> [p, s*kq*i2*e]
        asd = ai[:, j * EL : (j + 1) * EL]
        aim = (
            asd.reshape(KQ, 2, P, NS, NT)
            .transpose(2, 3, 0, 1, 4)
            .reshape(P, NS * KQ * 2 * NT)
        )
        in_maps.append(
            {
                "xq": np.ascontiguousarray(xim),
                "aq": np.ascontiguousarray(aim),
                "cvec": cvec,
                "cvec2": cvec2,
            }
        )
    return in_maps


def _run(x: np.ndarray, A: np.ndarray, trace: bool = False):
    from concourse import bass_utils

    nc = _cache.get("nc")
    if nc is None:
        nc = _build()
        _cache["nc"] = nc

    in_maps = _shard_inputs(np.asarray(x), np.asarray(A))
    res = bass_utils.run_bass_kernel_spmd(
        nc, in_maps, list(range(N_CORES)), trace=trace
    )
    out = np.empty((N_FULL, E_FULL), dtype=np.float32)
    for c in range(N_CORES):
        i, j = divmod(c, GRID_E)
        out[i * ML : (i + 1) * ML, j * EL : (j + 1) * EL] = res.results[c][
            "out"
        ].astype(np.float32)
    return out, res


def kernel(x: np.ndarray, A: np.ndarray) -> np.ndarray:
    out, _ = _run(x, A, trace=False)
    return out
